# revision 74
# baseline (speedup 1.0000x reference)
"""Trainium2 Bass kernel for a 2-layer mean-aggregation GraphSAGE GNN.

Strategy (8 NeuronCores, SPMD):
  - Nodes are assigned to (core, tile, slot) with degree balancing; each core
    owns 49 tiles x 128 slots = 6272 dst nodes and the ~100k edges into them.
  - Layer 1: the x[src] edge stream is a pure permutation of the input, so
    the host pre-lays it out in fp8 DoubleRow chunk order (256 edges/chunk)
    and the device streams it sequentially (no gathers).  Per chunk, a wide
    one-hot eq[e, slot] built on DVE feeds fp8 DoubleRow segment matmuls into
    PSUM; 1/deg is applied as a per-column scale at eviction.
    H^T = relu(W1_l @ mean^T + W1_r @ x^T + b1) via matmuls + fused ScalarE.
  - g = h @ W2_l^T computed per tile (node-major), written to DRAM and
    AllGather'd across cores (bf16, 2 supertile-aligned pieces pipelined
    behind stage A; lo <= 28672 rows for the int16 gather index limit).
  - Layer 2: per 128-edge chunk dma_gather g[src] rows (256B descriptors,
    8-chunk calls = SWDGE ring capacity), segment matmuls + W2_r @ H^T into
    separate PSUM, combined with 1/deg scale and b2 on DVE -> bf16 output
    shard.  Lo-group gathers for the first PRE supertiles are issued before
    the AG-hi trigger to fill that window without starving the collective.
Host does layout-only preprocessing (permutation, edge chunking, fp8 cast,
1/deg) and the final unshard/transpose.
"""

import functools
import numpy as np

N_CORES = 8
TILES = 49  # tiles per core
TILE = 128
SHARD = TILES * TILE  # 6272
SUPER = 7  # tiles per supertile (gather-call granularity)
N_SUPER = TILES // SUPER  # 7
LO_SUPERS = 4  # supertiles in the "lo" AllGather split
LO_ROWS = LO_SUPERS * SUPER * TILE  # 3584
HI_ROWS = SHARD - LO_ROWS  # 2688
SPLIT16 = 32768  # int16 index limit for g-table gathers
GCHUNKS = 8  # chunks (x128 idxs) per dma_gather call (ring limit: 65 descs/engine ok, 97+ crashes)
# AllGather pieces: supertile-aligned shard row ranges (2 pieces — per-call
# collective overhead makes finer splits a net loss).
Q_BOUNDS = [0, 4 * SUPER * TILE, SHARD]  # lo table 8*3584=28672 <= int16 max
NQ = 2
PRE = 2  # lo-group gather supertiles issued ahead of the AG-hi gate


def _ceil_div(a, b):
    return -(-a // b)


def _wrap_idxs(idx_flat):
    """Wrap a flat int16 index list into the [128, n/16] dma_gather layout:
    index i lives at [i%16, i//16], replicated across the 8 groups of 16
    partitions."""
    n = len(idx_flat)
    assert n % 16 == 0
    w = np.asarray(idx_flat, np.int16).reshape(n // 16, 16).T  # [16, n/16]
    return np.tile(w, (8, 1))  # [128, n/16]


def _preprocess(x, edge_index, n_nodes):
    """Index-only host preprocessing: node permutation, per-core edge chunk
    streams for both layers, degree reciprocals.  Returns a dict of
    per-core/shared arrays plus layout metadata."""
    src = np.asarray(edge_index[0], np.int64)
    dst = np.asarray(edge_index[1], np.int64)
    E = src.shape[0]

    deg = np.bincount(dst, minlength=n_nodes).astype(np.int64)
    rdeg = (1.0 / np.maximum(deg, 1)).astype(np.float32)

    # Degree-balanced permutation: sort nodes by degree desc, deal round-robin
    # over the 392 global tiles; node -> (core, tile, slot).
    order = np.argsort(-deg, kind="stable")
    g_tile = np.empty(n_nodes, np.int64)   # global tile of node
    g_slot = np.empty(n_nodes, np.int64)   # slot within tile
    n_gtiles = N_CORES * TILES
    idx = np.arange(n_nodes)
    g_tile[order] = idx % n_gtiles
    g_slot[order] = idx // n_gtiles
    core_of = g_tile // TILES
    tile_of = g_tile % TILES
    row_of = tile_of * TILE + g_slot  # row within core shard [0, SHARD)

    e_core = core_of[dst]
    e_tile = tile_of[dst]
    e_slot = g_slot[dst]

    # Layer-2 groups: by gathered-g row quarter (AllGather piece layout).
    s_core = core_of[src]
    s_row = row_of[src]
    qb = np.asarray(Q_BOUNDS)
    qsizes = qb[1:] - qb[:-1]
    l2_grp = np.digitize(s_row, qb[1:-1])  # 0..NQ-1
    l2_idx = s_core * qsizes[l2_grp] + (s_row - qb[l2_grp])

    def build_layer(grp, gidx, ngrp):
        """Compute per-(core,tile,group) edge lists; fixed chunk budgets CQ[g]
        (max over all cores/tiles); build idx/dstslot streams in supertile
        gather-call order."""
        counts = np.zeros((N_CORES, TILES, ngrp), np.int64)
        np.add.at(counts, (e_core, e_tile, grp), 1)
        CQ = [int(_ceil_div(counts[:, :, g].max(), TILE)) for g in range(ngrp)]
        # bucket edges
        key = (e_core * TILES + e_tile) * ngrp + grp
        eorder = np.argsort(key * (2 * E) + gidx, kind="stable")  # sorted by key then src for DMA locality
        sorted_key = key[eorder]
        starts = np.searchsorted(sorted_key, np.arange(N_CORES * TILES * ngrp))
        ends = np.searchsorted(sorted_key, np.arange(N_CORES * TILES * ngrp) + 1)

        import ml_dtypes
        NCHUNK = TILES * sum(CQ)
        idx_cols_per_chunk = TILE // 16  # 8
        idx_arr = np.zeros((N_CORES, 128, NCHUNK * idx_cols_per_chunk), np.int16)
        ds_arr = np.full((N_CORES, 128, NCHUNK), -1.0, np.float32)

        for c in range(N_CORES):
            flat_idx = np.zeros(NCHUNK * TILE, np.int16)
            gc = 0  # global chunk cursor within core stream
            for S in range(N_SUPER):
                for g in range(ngrp):
                    nch = CQ[g]
                    for t0 in range(SUPER):
                        t = S * SUPER + t0
                        k = ((c * TILES + t) * ngrp) + g
                        es = eorder[starts[k]:ends[k]]
                        n_e = len(es)
                        assert n_e <= nch * TILE
                        span = slice(gc * TILE, gc * TILE + n_e)
                        flat_idx[span] = gidx[es].astype(np.int16)
                        pp = np.arange(n_e)
                        ds_arr[c, pp % 128, gc + pp // 128] = e_slot[es]
                        gc += nch
            idx_arr[c] = _wrap_idxs(flat_idx)
        return dict(CQ=tuple(CQ), idx=idx_arr,
                    ds=ds_arr.astype(ml_dtypes.bfloat16))

    def build_stream():
        """Layer-1 host-prepped edge stream: per (core, tile) edge lists in
        uniform 256-edge (DoubleRow fp8) chunk layout; returns chunk budget,
        per-core src-id lists and half-split dstslot arrays."""
        counts = np.zeros((N_CORES, TILES), np.int64)
        np.add.at(counts, (e_core, e_tile), 1)
        CL = int(_ceil_div(counts.max(), 2 * TILE))
        key = e_core * TILES + e_tile
        eorder = np.argsort(key, kind="stable")
        sorted_key = key[eorder]
        starts = np.searchsorted(sorted_key, np.arange(N_CORES * TILES))
        ends = np.searchsorted(sorted_key, np.arange(N_CORES * TILES) + 1)
        NCHUNK = TILES * CL
        ds_arr = np.full((N_CORES, 128, NCHUNK * 2), -1.0, np.float32)
        srcs = np.zeros((N_CORES, NCHUNK, 2 * TILE), np.int64)
        for c in range(N_CORES):
            for t in range(TILES):
                k = c * TILES + t
                es = eorder[starts[k]:ends[k]]
                n_e = len(es)
                gc = t * CL
                pp = np.arange(n_e)
                srcs[c, gc + pp // 256, pp % 256] = src[es]
                ds_arr[c, pp % 128,
                       (gc + pp // 256) * 2 + (pp % 256) // 128] = e_slot[es]
        import ml_dtypes
        return dict(CL=CL, srcs=srcs, ds=ds_arr.astype(ml_dtypes.bfloat16))

    l1 = build_stream()
    l2 = build_layer(l2_grp, l2_idx, NQ)

    # Per-core x^T in slot order (zeros for pad slots).
    import ml_dtypes
    din = x.shape[1]
    xT = np.zeros((N_CORES, din, SHARD), np.float32)
    xT[core_of, :, row_of] = np.asarray(x, np.float32)  # fancy: for each node
    xT_bf = xT.astype(ml_dtypes.bfloat16)

    # Layer-1 pre-gathered edge stream in the [128, NCHUNK, 2, din] fp8
    # DoubleRow layout the chunk matmuls consume (pure host-side permutation
    # of x).
    x_f8 = np.asarray(x, np.float32).astype(ml_dtypes.float8_e4m3fn)
    nch1 = l1["srcs"].shape[1]
    m1 = np.ascontiguousarray(
        x_f8[l1["srcs"]].reshape(N_CORES, nch1, 2, 128, din)
        .transpose(0, 3, 1, 2, 4))

    # Per-core 1/deg row in slot order, replicated across 128 partitions
    # (per-column scale applied at PSUM eviction; pad slots get 0).
    rt_flat = np.zeros((N_CORES, SHARD), np.float32)
    rt_flat[core_of, row_of] = rdeg
    rt = np.ascontiguousarray(
        np.broadcast_to(rt_flat[:, None, :], (N_CORES, 128, SHARD))
    ).astype(ml_dtypes.bfloat16)

    meta = dict(l1=l1, l2=l2, xT=xT_bf, rt=rt, m1=m1,
                core_of=core_of, row_of=row_of)
    return meta


@functools.lru_cache(maxsize=2)
def _build_program(din, dh, dout, CL1, CQ,
                   do_cc=True, do_c=True, shared_g=True):
    """Build the SPMD Bass/Tile program.  All shapes static."""
    import concourse.bacc as bacc
    import concourse.mybir as mybir
    import concourse.tile as tile
    from concourse.library_config import mlp

    bf16 = mybir.dt.bfloat16
    f32 = mybir.dt.float32
    f8 = mybir.dt.float8e4
    i16 = mybir.dt.int16

    SCQ = sum(CQ)
    NC1 = TILES * CL1  # layer-1 chunks are 256 edges (fp8 DoubleRow)
    NC2 = TILES * SCQ
    W2 = NC2 * 8  # idx cols (TILE/16 per chunk)
    MW = max(2 * CL1, SCQ)  # eq one-hot tile chunk width
    QSIZES = [Q_BOUNDS[i + 1] - Q_BOUNDS[i] for i in range(NQ)]
    NA = SUPER * CQ[0]  # chunk capacity of an mA buffer
    NB = SUPER * CQ[1]  # chunk capacity of an mB buffer
    assert SUPER * CL1 <= NA  # stage A fp8 stream fits one mA buffer's bytes

    nc = bacc.Bacc("TRN2", target_bir_lowering=False, debug=False,
                   num_devices=N_CORES, num_swdge_queues=4)

    # ---- I/O tensors ----
    m1_d = nc.dram_tensor("m1", [128, NC1, 2, din], f8, kind="ExternalInput")
    xT_d = nc.dram_tensor("xT", [din, SHARD], bf16, kind="ExternalInput")
    idx2_d = nc.dram_tensor("idx2", [128, W2], i16, kind="ExternalInput")
    ds1_d = nc.dram_tensor("ds1", [128, NC1 * 2], bf16, kind="ExternalInput")
    ds2_d = nc.dram_tensor("ds2", [128, NC2], bf16, kind="ExternalInput")
    rt_d = nc.dram_tensor("rt", [128, SHARD], bf16, kind="ExternalInput")
    w1lT_d = nc.dram_tensor("w1lT", [din, dh], bf16, kind="ExternalInput")
    w1rT_d = nc.dram_tensor("w1rT", [din, dh], bf16, kind="ExternalInput")
    w2lT_d = nc.dram_tensor("w2lT", [128, dh // 128, dout], bf16, kind="ExternalInput")
    w2rT_d = nc.dram_tensor("w2rT", [128, dh // 128, dout], bf16, kind="ExternalInput")
    b1_d = nc.dram_tensor("b1", [128, dh // 128], f32, kind="ExternalInput")
    b2_d = nc.dram_tensor("b2", [128, 1], f32, kind="ExternalInput")
    iota_d = nc.dram_tensor("iota", [128, 128], bf16, kind="ExternalInput")
    iotaw_d = nc.dram_tensor("iotaw", [128, MW * 128], bf16, kind="ExternalInput")
    outT_d = nc.dram_tensor("outT", [dout, SHARD], bf16, kind="ExternalOutput")

    # internal DRAM: per-quarter local g and AllGather'd g tables
    _aspace = "Shared" if shared_g else None
    gl_q = [nc.dram_tensor(f"gl_q{i}", [QSIZES[i], dout], bf16)
            for i in range(NQ)]
    gf_q = [nc.dram_tensor(f"gf_q{i}", [N_CORES * QSIZES[i], dout], bf16,
                           addr_space=_aspace) for i in range(NQ)]

    NH = dh // 128  # h halves (2)

    with tile.TileContext(nc) as tc:
        with (
            tc.tile_pool(name="per", bufs=1) as per,       # persistent SBUF
            tc.tile_pool(name="gathA", bufs=PRE) as gpoolA,  # lo-group m bufs
            tc.tile_pool(name="gathB", bufs=2) as gpoolB,    # hi-group m bufs
            tc.tile_pool(name="rt", bufs=2) as rpool,      # eq one-hot tiles
            tc.tile_pool(name="mt", bufs=3) as mpool,      # meanT / evict tiles
            tc.tile_pool(name="stg", bufs=3) as spool,     # staging for DRAM writes
            tc.tile_pool(name="ps_seg", bufs=2, space="PSUM") as ps_seg,
            tc.tile_pool(name="ps_h", bufs=2, space="PSUM") as ps_h,
            tc.tile_pool(name="ps_g", bufs=2, space="PSUM") as ps_g,
            tc.tile_pool(name="ps_o", bufs=2, space="PSUM") as ps_o,
        ):
            # ---- persistent loads ----
            xT = per.tile([din, SHARD], bf16)
            idx2 = per.tile([128, W2], i16)
            ds1 = per.tile([128, NC1 * 2], bf16)
            ds2 = per.tile([128, NC2], bf16)
            rt = per.tile([128, SHARD], bf16)
            w1lT = per.tile([din, dh], bf16)
            w1rT = per.tile([din, dh], bf16)
            w2lT = per.tile([128, NH, dout], bf16)
            w2rT = per.tile([128, NH, dout], bf16)
            b1 = per.tile([128, NH], f32)
            b2 = per.tile([128, 1], f32)
            iota = per.tile([128, 128], bf16)
            iotaw = per.tile([128, MW, 128], bf16)
            HT = per.tile([128, NH, SHARD], bf16)

            for t_sb, t_dr in [(xT, xT_d), (idx2, idx2_d),
                               (ds1, ds1_d), (ds2, ds2_d), (rt, rt_d),
                               (w1lT, w1lT_d), (w1rT, w1rT_d),
                               (w2lT, w2lT_d), (w2rT, w2rT_d), (b1, b1_d),
                               (b2, b2_d), (iota, iota_d), (iotaw, iotaw_d)]:
                nc.sync.dma_start(t_sb[:], t_dr[:])

            nc.gpsimd.load_library(mlp)

            # ================= Stage A: layer 1 + H + g =================
            # The fp8 DoubleRow stream for a supertile (SUPER*CL1 chunks of
            # 256 edges, 256B each) fits inside one mA-shaped buffer's bytes;
            # stage C reuses the same pool for its lo gather group.
            nsc = SUPER * CL1
            for S in range(N_SUPER):
                mSa = gpoolA.tile([128, NA, din], bf16, tag="mA")
                nc.sync.dma_start(mSa[:, 0:nsc, :].bitcast(f8),
                                  m1_d[:, S * nsc:(S + 1) * nsc, :, :])
                for t0 in range(SUPER):
                    t = S * SUPER + t0
                    psS = ps_seg.tile([128, 128], f32, tag="psS")
                    eq1 = rpool.tile([128, 2 * CL1, 128], f8, tag="eq1")
                    nc.vector.tensor_tensor(
                        eq1[:],
                        ds1[:, t * 2 * CL1:(t + 1) * 2 * CL1, None].broadcast_to(
                            (128, 2 * CL1, 128)),
                        iotaw[:, 0:2 * CL1, :],
                        mybir.AluOpType.is_equal)
                    for k in range(CL1):
                        lhs3 = mSa[:, t0 * CL1 + k, :].bitcast(f8).rearrange(
                            "p (two f) -> p two f", two=2)
                        nc.tensor.matmul(psS[:], lhsT=lhs3,
                                         rhs=eq1[:, 2 * k:2 * k + 2, :],
                                         start=(k == 0), stop=(k == CL1 - 1),
                                         perf_mode=mybir.MatmulPerfMode.DoubleRow)
                    meanT = mpool.tile([128, 128], bf16, tag="meanT")
                    nc.vector.tensor_tensor(meanT[:], psS[:],
                                            rt[:, t * TILE:(t + 1) * TILE],
                                            mybir.AluOpType.mult)
                    # H^T halves
                    for j in range(NH):
                        psH = ps_h.tile([128, 128], f32, tag="psH")
                        nc.tensor.matmul(psH[:], lhsT=w1lT[:, j * 128:(j + 1) * 128],
                                         rhs=meanT[:], start=True, stop=False)
                        nc.tensor.matmul(psH[:], lhsT=w1rT[:, j * 128:(j + 1) * 128],
                                         rhs=xT[:, t * TILE:(t + 1) * TILE],
                                         start=False, stop=True)
                        nc.scalar.activation(HT[:, j, t * TILE:(t + 1) * TILE], psH[:],
                                             mybir.ActivationFunctionType.Relu,
                                             bias=b1[:, j:j + 1])
                    # g tile (node-major)
                    psG = ps_g.tile([128, 128], f32, tag="psG")
                    for j in range(NH):
                        nc.tensor.matmul(psG[:], lhsT=HT[:, j, t * TILE:(t + 1) * TILE],
                                         rhs=w2lT[:, j, :], start=(j == 0),
                                         stop=(j == NH - 1))
                    gT = spool.tile([128, dout], bf16, tag="gT")
                    nc.scalar.activation(gT[:], psG[:],
                                         mybir.ActivationFunctionType.Copy)
                    row = t * TILE
                    q = next(i for i in range(NQ)
                             if Q_BOUNDS[i] <= row < Q_BOUNDS[i + 1])
                    dst = gl_q[q][row - Q_BOUNDS[q]:row - Q_BOUNDS[q] + TILE, :]
                    nc.sync.dma_start(dst, gT[:])
                # AllGather piece as soon as its supertiles' g is written
                # (pipelines behind the rest of stage A).
                import concourse.mybir as _mb
                for q in range(NQ - 1):
                    if do_cc and S == Q_BOUNDS[q + 1] // (SUPER * TILE) - 1:
                        nc.gpsimd.collective_compute(
                            "AllGather", _mb.AluOpType.bypass,
                            replica_groups=[list(range(N_CORES))],
                            ins=[gl_q[q].ap().opt()], outs=[gf_q[q].ap().opt()])

            # ================= Stage C: layer 2 =================
            # Gathers grouped per (supertile, lo/hi); lo gathers for the first
            # PRE supertiles are issued before the AG-hi trigger so their Q7
            # generation and DMA drain fill the AG-hi completion window.  The
            # hi gather of supertile S is issued before the lo gather of
            # S+PRE (which waits on S's buffer) to avoid head-of-line
            # deadlock on the serial GpSimd queue.
            def _issue_c(S, g, bufs_by_S):
                col0 = S * SUPER * SCQ * 8
                if g == 0:
                    mC = gpoolA.tile([128, NA, dout], bf16, tag="mA")
                    bufs_by_S.setdefault(S, {})[0] = mC
                    nch, c0 = SUPER * CQ[0], col0
                else:
                    mC = gpoolB.tile([128, NB, dout], bf16, tag="mB")
                    bufs_by_S.setdefault(S, {})[1] = mC
                    nch, c0 = SUPER * CQ[1], col0 + SUPER * CQ[0] * 8
                for q0 in range(0, nch, GCHUNKS):
                    n = min(GCHUNKS, nch - q0)
                    nc.gpsimd.dma_gather(
                        mC[:, q0:q0 + n, :], gf_q[g][:],
                        idx2[:, c0 + q0 * 8:c0 + (q0 + n) * 8],
                        n * TILE, n * TILE, dout)

            _c_bufs = {}
            if do_c:
                for Sp in range(min(PRE, N_SUPER)):
                    _issue_c(Sp, 0, _c_bufs)
            if do_cc:
                nc.gpsimd.collective_compute(
                    "AllGather", _mb.AluOpType.bypass,
                    replica_groups=[list(range(N_CORES))],
                    ins=[gl_q[NQ - 1].ap().opt()], outs=[gf_q[NQ - 1].ap().opt()])
            if do_c:
                _issue_c(0, 1, _c_bufs)
            for S in (range(N_SUPER) if do_c else []):
                if S + 1 < N_SUPER:
                    _issue_c(S + 1, 1, _c_bufs)
                if S + PRE < N_SUPER:
                    _issue_c(S + PRE, 0, _c_bufs)
                gc0 = S * SUPER * SCQ
                for t0 in range(SUPER):
                    t = S * SUPER + t0
                    psS2 = ps_seg.tile([128, 128], f32, tag="psS")
                    nchunks = SCQ
                    eq = rpool.tile([128, MW, 128], bf16, tag="eq")
                    ci = 0
                    for g in range(NQ):
                        CC = CQ[g]
                        if CC == 0:
                            continue
                        mC = _c_bufs[S][g]
                        gcs = gc0 + SUPER * sum(CQ[:g]) + t0 * CC
                        nc.vector.tensor_tensor(
                            eq[:, ci:ci + CC, :],
                            ds2[:, gcs:gcs + CC, None].broadcast_to((128, CC, 128)),
                            iotaw[:, 0:CC, :],
                            mybir.AluOpType.is_equal)
                        for k in range(CC):
                            nc.tensor.matmul(psS2[:],
                                             lhsT=mC[:, t0 * CC + k, :],
                                             rhs=eq[:, ci, :], start=(ci == 0),
                                             stop=(ci == nchunks - 1))
                            ci += 1
                    psO = ps_o.tile([128, 128], f32, tag="psO")
                    for j in range(NH):
                        nc.tensor.matmul(psO[:], lhsT=w2rT[:, j, :],
                                         rhs=HT[:, j, t * TILE:(t + 1) * TILE],
                                         start=(j == 0), stop=(j == NH - 1))
                    tmp = mpool.tile([128, 128], f32, tag="tmp")
                    nc.vector.tensor_tensor(tmp[:], psS2[:],
                                            rt[:, t * TILE:(t + 1) * TILE],
                                            mybir.AluOpType.mult)
                    oT = spool.tile([128, 128], bf16, tag="oT")
                    nc.vector.scalar_tensor_tensor(
                        oT[:], tmp[:], b2[:, 0:1], psO[:],
                        mybir.AluOpType.add, mybir.AluOpType.add)
                    nc.sync.dma_start(
                        outT_d[:, t * TILE:(t + 1) * TILE], oT[:])

    # Align each gather's SWDGE queue with the DMASW sem lane Tile assigned
    # (sem lane L is locked to one queue; use queue = L % num_queues).
    import re as _re
    n_fix = 0
    for bb in nc.main_func.blocks:
        for ins in bb.instructions:
            if isinstance(ins, mybir.InstDMAGatherAnt):
                lane = None
                si = ins.sync_info
                if si is not None:
                    for upd in list(si.on_update):
                        m = _re.match(r"DMASW(\d+)", getattr(upd, "ant_name", None) or "")
                        if m:
                            lane = int(m.group(1))
                if lane is not None:
                    ins.queue_num = lane % 4
                    n_fix += 1
    nc.compile()
    return nc


def kernel(x, edge_index, W1_l, b1_l, W1_r, W2_l, b2_l, W2_r):
    import ml_dtypes
    from concourse.bass_utils import run_bass_kernel_spmd

    x = np.asarray(x, np.float32)
    n_nodes, din = x.shape
    dh = W1_l.shape[0]
    dout = W2_l.shape[0]

    meta = _preprocess(x, edge_index, n_nodes)
    l1, l2 = meta["l1"], meta["l2"]

    nc = _build_program(din, dh, dout, l1["CL"], l2["CQ"])

    bf = ml_dtypes.bfloat16
    w1lT = np.ascontiguousarray(np.asarray(W1_l, np.float32).T).astype(bf)  # [din, dh]
    w1rT = np.ascontiguousarray(np.asarray(W1_r, np.float32).T).astype(bf)
    # [dh, dout] -> [128, dh//128, dout]
    w2lT = np.ascontiguousarray(np.asarray(W2_l, np.float32).T).reshape(
        dh // 128, 128, dout).transpose(1, 0, 2).astype(bf)
    w2rT = np.ascontiguousarray(np.asarray(W2_r, np.float32).T).reshape(
        dh // 128, 128, dout).transpose(1, 0, 2).astype(bf)
    b1 = np.ascontiguousarray(
        np.asarray(b1_l, np.float32).reshape(dh // 128, 128).T)  # [128, nh]
    b2 = np.asarray(b2_l, np.float32).reshape(128, 1)
    iota = np.tile(np.arange(128, dtype=np.float32), (128, 1)).astype(bf)
    MW = max(2 * l1["CL"], sum(l2["CQ"]))
    iotaw = np.tile(iota, (1, MW))

    in_maps = []
    for c in range(N_CORES):
        in_maps.append({
            "m1": meta["m1"][c], "xT": meta["xT"][c], "rt": meta["rt"][c],
            "idx2": l2["idx"][c],
            "ds1": l1["ds"][c], "ds2": l2["ds"][c],
            "w1lT": w1lT, "w1rT": w1rT, "w2lT": w2lT, "w2rT": w2rT,
            "b1": b1, "b2": b2, "iota": iota, "iotaw": iotaw,
        })

    res = run_bass_kernel_spmd(nc, in_maps, list(range(N_CORES)))

    out = np.empty((n_nodes, dout), np.float32)
    core_of, row_of = meta["core_of"], meta["row_of"]
    outTs = np.stack([np.asarray(res.results[c]["outT"], np.float32)
                      for c in range(N_CORES)])  # [8, dout, SHARD]
    out[:, :] = outTs[core_of, :, row_of]
    return out



# revision 75
# speedup vs baseline: 1.0918x; 1.0918x over previous
"""Trainium2 Bass kernel for a 2-layer mean-aggregation GraphSAGE GNN.

Strategy (8 NeuronCores, SPMD):
  - Nodes are assigned to (core, tile, slot) with degree balancing; each core
    owns 49 tiles x 128 slots = 6272 dst nodes and the ~100k edges into them.
  - Layer 1: the x[src] edge stream is a pure permutation of the input, so
    the host pre-lays it out in fp8 DoubleRow chunk order (256 edges/chunk)
    and the device streams it sequentially (no gathers).  Per chunk, a wide
    one-hot eq[e, slot] built on DVE feeds fp8 DoubleRow segment matmuls into
    PSUM; 1/deg is applied as a per-column scale at eviction.
    H^T = relu(W1_l @ mean^T + W1_r @ x^T + b1) via matmuls + fused ScalarE.
  - g = h @ W2_l^T computed per tile (node-major), written to DRAM and
    AllGather'd across cores (bf16, 2 supertile-aligned pieces pipelined
    behind stage A; lo <= 28672 rows for the int16 gather index limit).
  - Layer 2: per 128-edge chunk dma_gather g[src] rows (256B descriptors,
    8-chunk calls = SWDGE ring capacity), segment matmuls + W2_r @ H^T into
    separate PSUM, combined with 1/deg scale and b2 on DVE -> bf16 output
    shard.  Lo-group gathers for the first PRE supertiles are issued before
    the AG-hi trigger to fill that window without starving the collective.
Host does layout-only preprocessing (permutation, edge chunking, fp8 cast,
1/deg) and the final unshard/transpose.
"""

import functools
import numpy as np

N_CORES = 8
TILES = 49  # tiles per core
TILE = 128
SHARD = TILES * TILE  # 6272
SUPER = 7  # tiles per supertile (gather-call granularity)
N_SUPER = TILES // SUPER  # 7
LO_SUPERS = 4  # supertiles in the "lo" AllGather split
LO_ROWS = LO_SUPERS * SUPER * TILE  # 3584
HI_ROWS = SHARD - LO_ROWS  # 2688
SPLIT16 = 32768  # int16 index limit for g-table gathers
GCHUNKS = 8  # chunks (x128 idxs) per dma_gather call (ring limit: 65 descs/engine ok, 97+ crashes)
# AllGather pieces: supertile-aligned shard row ranges (2 pieces — per-call
# collective overhead makes finer splits a net loss).
Q_BOUNDS = [0, 4 * SUPER * TILE, SHARD]  # lo table 8*3584=28672 <= int16 max
NQ = 2
PRE = 2  # lo-group gather supertiles issued ahead of the AG-hi gate


def _ceil_div(a, b):
    return -(-a // b)


def _wrap_idxs(idx_flat):
    """Wrap a flat int16 index list into the [128, n/16] dma_gather layout:
    index i lives at [i%16, i//16], replicated across the 8 groups of 16
    partitions."""
    n = len(idx_flat)
    assert n % 16 == 0
    w = np.asarray(idx_flat, np.int16).reshape(n // 16, 16).T  # [16, n/16]
    return np.tile(w, (8, 1))  # [128, n/16]


def _preprocess(x, edge_index, n_nodes):
    """Index-only host preprocessing: node permutation, per-core edge chunk
    streams for both layers, degree reciprocals.  Returns a dict of
    per-core/shared arrays plus layout metadata."""
    src = np.asarray(edge_index[0], np.int64)
    dst = np.asarray(edge_index[1], np.int64)
    E = src.shape[0]

    deg = np.bincount(dst, minlength=n_nodes).astype(np.int64)
    rdeg = (1.0 / np.maximum(deg, 1)).astype(np.float32)

    # Degree-balanced permutation: sort nodes by degree desc, deal round-robin
    # over the 392 global tiles; node -> (core, tile, slot).
    order = np.argsort(-deg, kind="stable")
    g_tile = np.empty(n_nodes, np.int64)   # global tile of node
    g_slot = np.empty(n_nodes, np.int64)   # slot within tile
    n_gtiles = N_CORES * TILES
    idx = np.arange(n_nodes)
    g_tile[order] = idx % n_gtiles
    g_slot[order] = idx // n_gtiles
    core_of = g_tile // TILES
    tile_of = g_tile % TILES
    row_of = tile_of * TILE + g_slot  # row within core shard [0, SHARD)

    e_core = core_of[dst]
    e_tile = tile_of[dst]
    e_slot = g_slot[dst]

    # Layer-2 groups: by gathered-g row quarter (AllGather piece layout).
    s_core = core_of[src]
    s_row = row_of[src]
    qb = np.asarray(Q_BOUNDS)
    qsizes = qb[1:] - qb[:-1]
    l2_grp = np.digitize(s_row, qb[1:-1])  # 0..NQ-1
    l2_idx = s_core * qsizes[l2_grp] + (s_row - qb[l2_grp])

    def build_layer(grp, gidx, ngrp):
        """Compute per-(core,tile,group) edge lists; fixed chunk budgets CQ[g]
        (max over all cores/tiles); build idx/dstslot streams in supertile
        gather-call order."""
        counts = np.zeros((N_CORES, TILES, ngrp), np.int64)
        np.add.at(counts, (e_core, e_tile, grp), 1)
        CQ = [int(_ceil_div(counts[:, :, g].max(), TILE)) for g in range(ngrp)]
        # bucket edges
        key = (e_core * TILES + e_tile) * ngrp + grp
        eorder = np.argsort(key * (2 * E) + gidx, kind="stable")  # sorted by key then src for DMA locality
        sorted_key = key[eorder]
        starts = np.searchsorted(sorted_key, np.arange(N_CORES * TILES * ngrp))
        ends = np.searchsorted(sorted_key, np.arange(N_CORES * TILES * ngrp) + 1)

        import ml_dtypes
        NCHUNK = TILES * sum(CQ)
        idx_cols_per_chunk = TILE // 16  # 8
        idx_arr = np.zeros((N_CORES, 128, NCHUNK * idx_cols_per_chunk), np.int16)
        ds_arr = np.full((N_CORES, 128, NCHUNK), -1.0, np.float32)

        for c in range(N_CORES):
            flat_idx = np.zeros(NCHUNK * TILE, np.int16)
            gc = 0  # global chunk cursor within core stream
            for S in range(N_SUPER):
                for g in range(ngrp):
                    nch = CQ[g]
                    for t0 in range(SUPER):
                        t = S * SUPER + t0
                        k = ((c * TILES + t) * ngrp) + g
                        es = eorder[starts[k]:ends[k]]
                        n_e = len(es)
                        assert n_e <= nch * TILE
                        span = slice(gc * TILE, gc * TILE + n_e)
                        flat_idx[span] = gidx[es].astype(np.int16)
                        pp = np.arange(n_e)
                        ds_arr[c, pp % 128, gc + pp // 128] = e_slot[es]
                        gc += nch
            idx_arr[c] = _wrap_idxs(flat_idx)
        return dict(CQ=tuple(CQ), idx=idx_arr,
                    ds=ds_arr.astype(ml_dtypes.bfloat16))

    def build_stream():
        """Layer-1 host-prepped edge stream: per (core, tile) edge lists in
        uniform 256-edge (DoubleRow fp8) chunk layout; returns chunk budget,
        per-core src-id lists and half-split dstslot arrays."""
        counts = np.zeros((N_CORES, TILES), np.int64)
        np.add.at(counts, (e_core, e_tile), 1)
        CL = int(_ceil_div(counts.max(), 2 * TILE))
        key = e_core * TILES + e_tile
        eorder = np.argsort(key, kind="stable")
        sorted_key = key[eorder]
        starts = np.searchsorted(sorted_key, np.arange(N_CORES * TILES))
        ends = np.searchsorted(sorted_key, np.arange(N_CORES * TILES) + 1)
        NCHUNK = TILES * CL
        ds_arr = np.full((N_CORES, 128, NCHUNK * 2), -1.0, np.float32)
        srcs = np.zeros((N_CORES, NCHUNK, 2 * TILE), np.int64)
        for c in range(N_CORES):
            for t in range(TILES):
                k = c * TILES + t
                es = eorder[starts[k]:ends[k]]
                n_e = len(es)
                gc = t * CL
                pp = np.arange(n_e)
                srcs[c, gc + pp // 256, pp % 256] = src[es]
                ds_arr[c, pp % 128,
                       (gc + pp // 256) * 2 + (pp % 256) // 128] = e_slot[es]
        import ml_dtypes
        return dict(CL=CL, srcs=srcs, ds=ds_arr.astype(ml_dtypes.bfloat16))

    l1 = build_stream()
    l2 = build_layer(l2_grp, l2_idx, NQ)

    # Per-core x^T in slot order (zeros for pad slots).
    import ml_dtypes
    din = x.shape[1]
    xT = np.zeros((N_CORES, din, SHARD), np.float32)
    xT[core_of, :, row_of] = np.asarray(x, np.float32)  # fancy: for each node
    xT_bf = xT.astype(ml_dtypes.bfloat16)

    # Layer-1 pre-gathered edge stream in the [128, NCHUNK, 2, din] fp8
    # DoubleRow layout the chunk matmuls consume (pure host-side permutation
    # of x).
    x_f8 = np.asarray(x, np.float32).astype(ml_dtypes.float8_e4m3fn)
    nch1 = l1["srcs"].shape[1]
    m1 = np.ascontiguousarray(
        x_f8[l1["srcs"]].reshape(N_CORES, nch1, 2, 128, din)
        .transpose(0, 3, 1, 2, 4))

    # Per-core 1/deg row in slot order, replicated across 128 partitions
    # (per-column scale applied at PSUM eviction; pad slots get 0).
    rt_flat = np.zeros((N_CORES, SHARD), np.float32)
    rt_flat[core_of, row_of] = rdeg
    rt = np.ascontiguousarray(
        np.broadcast_to(rt_flat[:, None, :], (N_CORES, 128, SHARD))
    ).astype(ml_dtypes.bfloat16)

    meta = dict(l1=l1, l2=l2, xT=xT_bf, rt=rt, m1=m1,
                core_of=core_of, row_of=row_of)
    return meta


@functools.lru_cache(maxsize=2)
def _build_program(din, dh, dout, CL1, CQ,
                   do_cc=True, do_c=True, shared_g=True):
    """Build the SPMD Bass/Tile program.  All shapes static."""
    import concourse.bacc as bacc
    import concourse.mybir as mybir
    import concourse.tile as tile
    from concourse.library_config import mlp

    bf16 = mybir.dt.bfloat16
    f32 = mybir.dt.float32
    f8 = mybir.dt.float8e4
    i16 = mybir.dt.int16

    SCQ = sum(CQ)
    NC1 = TILES * CL1  # layer-1 chunks are 256 edges (fp8 DoubleRow)
    NC2 = TILES * SCQ
    W2 = NC2 * 8  # idx cols (TILE/16 per chunk)
    MW = max(2 * CL1, SCQ)  # eq one-hot tile chunk width
    QSIZES = [Q_BOUNDS[i + 1] - Q_BOUNDS[i] for i in range(NQ)]
    NA = SUPER * CQ[0]  # chunk capacity of an mA buffer
    NB = SUPER * CQ[1]  # chunk capacity of an mB buffer
    assert SUPER * CL1 <= NA  # stage A fp8 stream fits one mA buffer's bytes

    nc = bacc.Bacc("TRN2", target_bir_lowering=False, debug=False,
                   num_devices=N_CORES, num_swdge_queues=4)

    # ---- I/O tensors ----
    m1_d = nc.dram_tensor("m1", [128, NC1, 2, din], f8, kind="ExternalInput")
    xT_d = nc.dram_tensor("xT", [din, SHARD], bf16, kind="ExternalInput")
    idx2_d = nc.dram_tensor("idx2", [128, W2], i16, kind="ExternalInput")
    ds1_d = nc.dram_tensor("ds1", [128, NC1 * 2], bf16, kind="ExternalInput")
    ds2_d = nc.dram_tensor("ds2", [128, NC2], bf16, kind="ExternalInput")
    rt_d = nc.dram_tensor("rt", [128, SHARD], bf16, kind="ExternalInput")
    w1lT_d = nc.dram_tensor("w1lT", [din, dh], bf16, kind="ExternalInput")
    w1rT_d = nc.dram_tensor("w1rT", [din, dh], bf16, kind="ExternalInput")
    w2lT_d = nc.dram_tensor("w2lT", [128, dh // 128, dout], bf16, kind="ExternalInput")
    w2rT_d = nc.dram_tensor("w2rT", [128, dh // 128, dout], bf16, kind="ExternalInput")
    b1_d = nc.dram_tensor("b1", [128, dh // 128], f32, kind="ExternalInput")
    b2_d = nc.dram_tensor("b2", [128, 1], f32, kind="ExternalInput")
    iota_d = nc.dram_tensor("iota", [128, 128], bf16, kind="ExternalInput")
    iotaw_d = nc.dram_tensor("iotaw", [128, MW * 128], bf16, kind="ExternalInput")
    outT_d = nc.dram_tensor("outT", [dout, SHARD], bf16, kind="ExternalOutput")

    # internal DRAM: per-quarter local g and AllGather'd g tables
    _aspace = "Shared" if shared_g else None
    gl_q = [nc.dram_tensor(f"gl_q{i}", [QSIZES[i], dout], bf16)
            for i in range(NQ)]
    gf_q = [nc.dram_tensor(f"gf_q{i}", [N_CORES * QSIZES[i], dout], bf16,
                           addr_space=_aspace) for i in range(NQ)]

    NH = dh // 128  # h halves (2)

    with tile.TileContext(nc) as tc:
        with (
            tc.tile_pool(name="per", bufs=1) as per,       # persistent SBUF
            tc.tile_pool(name="gathA", bufs=PRE) as gpoolA,  # lo-group m bufs
            tc.tile_pool(name="gathB", bufs=2) as gpoolB,    # hi-group m bufs
            tc.tile_pool(name="rt", bufs=2) as rpool,      # eq one-hot tiles
            tc.tile_pool(name="mt", bufs=3) as mpool,      # meanT / evict tiles
            tc.tile_pool(name="stg", bufs=3) as spool,     # staging for DRAM writes
            tc.tile_pool(name="ps_seg", bufs=2, space="PSUM") as ps_seg,
            tc.tile_pool(name="ps_h", bufs=2, space="PSUM") as ps_h,
            tc.tile_pool(name="ps_g", bufs=2, space="PSUM") as ps_g,
            tc.tile_pool(name="ps_o", bufs=2, space="PSUM") as ps_o,
        ):
            # ---- persistent loads ----
            xT = per.tile([din, SHARD], bf16)
            idx2 = per.tile([128, W2], i16)
            ds1 = per.tile([128, NC1 * 2], bf16)
            ds2 = per.tile([128, NC2], bf16)
            rt = per.tile([128, SHARD], bf16)
            w1lT = per.tile([din, dh], bf16)
            w1rT = per.tile([din, dh], bf16)
            w2lT = per.tile([128, NH, dout], bf16)
            w2rT = per.tile([128, NH, dout], bf16)
            b1 = per.tile([128, NH], f32)
            b2 = per.tile([128, 1], f32)
            iota = per.tile([128, 128], bf16)
            iotaw = per.tile([128, MW, 128], bf16)
            HT = per.tile([128, NH, SHARD], bf16)

            for t_sb, t_dr in [(xT, xT_d), (idx2, idx2_d),
                               (ds1, ds1_d), (ds2, ds2_d), (rt, rt_d),
                               (w1lT, w1lT_d), (w1rT, w1rT_d),
                               (w2lT, w2lT_d), (w2rT, w2rT_d), (b1, b1_d),
                               (b2, b2_d), (iota, iota_d), (iotaw, iotaw_d)]:
                nc.sync.dma_start(t_sb[:], t_dr[:])

            nc.gpsimd.load_library(mlp)

            # ================= Stage A: layer 1 + H + g =================
            # The fp8 DoubleRow stream for a supertile (SUPER*CL1 chunks of
            # 256 edges, 256B each) fits inside one mA-shaped buffer's bytes;
            # stage C reuses the same pool for its lo gather group.
            nsc = SUPER * CL1
            for S in range(N_SUPER):
                mSa = gpoolA.tile([128, NA, din], bf16, tag="mA")
                nc.sync.dma_start(mSa[:, 0:nsc, :].bitcast(f8),
                                  m1_d[:, S * nsc:(S + 1) * nsc, :, :])
                for t0 in range(SUPER):
                    t = S * SUPER + t0
                    psS = ps_seg.tile([128, 128], f32, tag="psS")
                    eq1 = rpool.tile([128, 2 * CL1, 128], f8, tag="eq1")
                    nc.vector.tensor_tensor(
                        eq1[:],
                        ds1[:, t * 2 * CL1:(t + 1) * 2 * CL1, None].broadcast_to(
                            (128, 2 * CL1, 128)),
                        iotaw[:, 0:2 * CL1, :],
                        mybir.AluOpType.is_equal)
                    for k in range(CL1):
                        lhs3 = mSa[:, t0 * CL1 + k, :].bitcast(f8).rearrange(
                            "p (two f) -> p two f", two=2)
                        nc.tensor.matmul(psS[:], lhsT=lhs3,
                                         rhs=eq1[:, 2 * k:2 * k + 2, :],
                                         start=(k == 0), stop=(k == CL1 - 1),
                                         perf_mode=mybir.MatmulPerfMode.DoubleRow)
                    meanT = mpool.tile([128, 128], bf16, tag="meanT")
                    nc.vector.tensor_tensor(meanT[:], psS[:],
                                            rt[:, t * TILE:(t + 1) * TILE],
                                            mybir.AluOpType.mult)
                    # H^T halves
                    for j in range(NH):
                        psH = ps_h.tile([128, 128], f32, tag="psH")
                        nc.tensor.matmul(psH[:], lhsT=w1lT[:, j * 128:(j + 1) * 128],
                                         rhs=meanT[:], start=True, stop=False)
                        nc.tensor.matmul(psH[:], lhsT=w1rT[:, j * 128:(j + 1) * 128],
                                         rhs=xT[:, t * TILE:(t + 1) * TILE],
                                         start=False, stop=True)
                        nc.scalar.activation(HT[:, j, t * TILE:(t + 1) * TILE], psH[:],
                                             mybir.ActivationFunctionType.Relu,
                                             bias=b1[:, j:j + 1])
                    # g tile (node-major)
                    psG = ps_g.tile([128, 128], f32, tag="psG")
                    for j in range(NH):
                        nc.tensor.matmul(psG[:], lhsT=HT[:, j, t * TILE:(t + 1) * TILE],
                                         rhs=w2lT[:, j, :], start=(j == 0),
                                         stop=(j == NH - 1))
                    gT = spool.tile([128, dout], bf16, tag="gT")
                    nc.scalar.activation(gT[:], psG[:],
                                         mybir.ActivationFunctionType.Copy)
                    row = t * TILE
                    q = next(i for i in range(NQ)
                             if Q_BOUNDS[i] <= row < Q_BOUNDS[i + 1])
                    dst = gl_q[q][row - Q_BOUNDS[q]:row - Q_BOUNDS[q] + TILE, :]
                    nc.sync.dma_start(dst, gT[:])
                # AllGather piece as soon as its supertiles' g is written
                # (pipelines behind the rest of stage A).
                import concourse.mybir as _mb
                for q in range(NQ - 1):
                    if do_cc and S == Q_BOUNDS[q + 1] // (SUPER * TILE) - 1:
                        nc.gpsimd.collective_compute(
                            "AllGather", _mb.AluOpType.bypass,
                            replica_groups=[list(range(N_CORES))],
                            ins=[gl_q[q].ap().opt()], outs=[gf_q[q].ap().opt()])

            # ================= Stage C: layer 2 =================
            # Gathers grouped per (supertile, lo/hi); lo gathers for the first
            # PRE supertiles are issued before the AG-hi trigger so their Q7
            # generation and DMA drain fill the AG-hi completion window.  The
            # hi gather of supertile S is issued before the lo gather of
            # S+PRE (which waits on S's buffer) to avoid head-of-line
            # deadlock on the serial GpSimd queue.
            def _issue_c(S, g, bufs_by_S):
                col0 = S * SUPER * SCQ * 8
                if g == 0:
                    mC = gpoolA.tile([128, NA, dout], bf16, tag="mA")
                    bufs_by_S.setdefault(S, {})[0] = mC
                    nch, c0 = SUPER * CQ[0], col0
                else:
                    mC = gpoolB.tile([128, NB, dout], bf16, tag="mB")
                    bufs_by_S.setdefault(S, {})[1] = mC
                    nch, c0 = SUPER * CQ[1], col0 + SUPER * CQ[0] * 8
                for q0 in range(0, nch, GCHUNKS):
                    n = min(GCHUNKS, nch - q0)
                    nc.gpsimd.dma_gather(
                        mC[:, q0:q0 + n, :], gf_q[g][:],
                        idx2[:, c0 + q0 * 8:c0 + (q0 + n) * 8],
                        n * TILE, n * TILE, dout)

            _c_bufs = {}
            if do_c:
                for Sp in range(min(PRE, N_SUPER)):
                    _issue_c(Sp, 0, _c_bufs)
            if do_cc:
                nc.gpsimd.collective_compute(
                    "AllGather", _mb.AluOpType.bypass,
                    replica_groups=[list(range(N_CORES))],
                    ins=[gl_q[NQ - 1].ap().opt()], outs=[gf_q[NQ - 1].ap().opt()])
            for S in (range(N_SUPER) if do_c else []):
                _issue_c(S, 1, _c_bufs)
                if S + PRE < N_SUPER:
                    _issue_c(S + PRE, 0, _c_bufs)
                gc0 = S * SUPER * SCQ
                for t0 in range(SUPER):
                    t = S * SUPER + t0
                    psS2 = ps_seg.tile([128, 128], f32, tag="psS")
                    nchunks = SCQ
                    eq = rpool.tile([128, MW, 128], bf16, tag="eq")
                    ci = 0
                    for g in range(NQ):
                        CC = CQ[g]
                        if CC == 0:
                            continue
                        mC = _c_bufs[S][g]
                        gcs = gc0 + SUPER * sum(CQ[:g]) + t0 * CC
                        nc.vector.tensor_tensor(
                            eq[:, ci:ci + CC, :],
                            ds2[:, gcs:gcs + CC, None].broadcast_to((128, CC, 128)),
                            iotaw[:, 0:CC, :],
                            mybir.AluOpType.is_equal)
                        for k in range(CC):
                            nc.tensor.matmul(psS2[:],
                                             lhsT=mC[:, t0 * CC + k, :],
                                             rhs=eq[:, ci, :], start=(ci == 0),
                                             stop=(ci == nchunks - 1))
                            ci += 1
                    psO = ps_o.tile([128, 128], f32, tag="psO")
                    for j in range(NH):
                        nc.tensor.matmul(psO[:], lhsT=w2rT[:, j, :],
                                         rhs=HT[:, j, t * TILE:(t + 1) * TILE],
                                         start=(j == 0), stop=(j == NH - 1))
                    tmp = mpool.tile([128, 128], f32, tag="tmp")
                    nc.vector.tensor_tensor(tmp[:], psS2[:],
                                            rt[:, t * TILE:(t + 1) * TILE],
                                            mybir.AluOpType.mult)
                    oT = spool.tile([128, 128], bf16, tag="oT")
                    nc.vector.scalar_tensor_tensor(
                        oT[:], tmp[:], b2[:, 0:1], psO[:],
                        mybir.AluOpType.add, mybir.AluOpType.add)
                    nc.sync.dma_start(
                        outT_d[:, t * TILE:(t + 1) * TILE], oT[:])

    # Align each gather's SWDGE queue with the DMASW sem lane Tile assigned
    # (sem lane L is locked to one queue; use queue = L % num_queues).
    import re as _re
    n_fix = 0
    for bb in nc.main_func.blocks:
        for ins in bb.instructions:
            if isinstance(ins, mybir.InstDMAGatherAnt):
                lane = None
                si = ins.sync_info
                if si is not None:
                    for upd in list(si.on_update):
                        m = _re.match(r"DMASW(\d+)", getattr(upd, "ant_name", None) or "")
                        if m:
                            lane = int(m.group(1))
                if lane is not None:
                    ins.queue_num = lane % 4
                    n_fix += 1
    nc.compile()
    return nc


def kernel(x, edge_index, W1_l, b1_l, W1_r, W2_l, b2_l, W2_r):
    import ml_dtypes
    from concourse.bass_utils import run_bass_kernel_spmd

    x = np.asarray(x, np.float32)
    n_nodes, din = x.shape
    dh = W1_l.shape[0]
    dout = W2_l.shape[0]

    meta = _preprocess(x, edge_index, n_nodes)
    l1, l2 = meta["l1"], meta["l2"]

    nc = _build_program(din, dh, dout, l1["CL"], l2["CQ"])

    bf = ml_dtypes.bfloat16
    w1lT = np.ascontiguousarray(np.asarray(W1_l, np.float32).T).astype(bf)  # [din, dh]
    w1rT = np.ascontiguousarray(np.asarray(W1_r, np.float32).T).astype(bf)
    # [dh, dout] -> [128, dh//128, dout]
    w2lT = np.ascontiguousarray(np.asarray(W2_l, np.float32).T).reshape(
        dh // 128, 128, dout).transpose(1, 0, 2).astype(bf)
    w2rT = np.ascontiguousarray(np.asarray(W2_r, np.float32).T).reshape(
        dh // 128, 128, dout).transpose(1, 0, 2).astype(bf)
    b1 = np.ascontiguousarray(
        np.asarray(b1_l, np.float32).reshape(dh // 128, 128).T)  # [128, nh]
    b2 = np.asarray(b2_l, np.float32).reshape(128, 1)
    iota = np.tile(np.arange(128, dtype=np.float32), (128, 1)).astype(bf)
    MW = max(2 * l1["CL"], sum(l2["CQ"]))
    iotaw = np.tile(iota, (1, MW))

    in_maps = []
    for c in range(N_CORES):
        in_maps.append({
            "m1": meta["m1"][c], "xT": meta["xT"][c], "rt": meta["rt"][c],
            "idx2": l2["idx"][c],
            "ds1": l1["ds"][c], "ds2": l2["ds"][c],
            "w1lT": w1lT, "w1rT": w1rT, "w2lT": w2lT, "w2rT": w2rT,
            "b1": b1, "b2": b2, "iota": iota, "iotaw": iotaw,
        })

    res = run_bass_kernel_spmd(nc, in_maps, list(range(N_CORES)))

    out = np.empty((n_nodes, dout), np.float32)
    core_of, row_of = meta["core_of"], meta["row_of"]
    outTs = np.stack([np.asarray(res.results[c]["outT"], np.float32)
                      for c in range(N_CORES)])  # [8, dout, SHARD]
    out[:, :] = outTs[core_of, :, row_of]
    return out

